# revision 8
# baseline (speedup 1.0000x reference)
"""Trainium2 Bass kernel for nn_NeighborAttention.

Key algebraic structure exploited: the attention query is a single
mean-pooled neighbor vector per batch, broadcast over the sequence.
Hence the [seq, seq] attention collapses to one weight vector per head
([nh, seq]) and the whole attention output is a single vector per batch
added to every row of x before the final LayerNorm.  The k/v
projections are never materialized: scores are computed as
x @ (q^T kw) and the value path as (w @ x) @ vw^T, reducing compute
from ~34 GFLOP to ~0.6 GFLOP.  Sharding: data-parallel over batch
(one batch element per NeuronCore, 8 cores).

Precision plan (output gate is 2e-2 relative error; final output is
dominated by LayerNorm(x + small-vector), so the attention path has
~100x error headroom):
  - residual x copy + output: bf16
  - all four weight matrices + x^T (scores/Sxv operand): fp8 e4m3,
    DoubleRow matmuls (2 contraction tiles per instruction)
  - softmax weights: bf16 (values are 1 +- 0.06; fp8 would flush the
    variation), so the pooled matmul runs bf16 against the residual
    x copy
  - fp8 operands carry power-of-2 scales chosen to sit in e4m3's
    normal range; all scales are folded into PSUM->SBUF copies.

Scheduling notes (from HW traces):
  - GpSimd is never used: its per-op ucode library swaps cost ~7us.
  - ACT runs only table-stable work per phase (Exp mid, Sqrt+Identity
    tail, with dummy activations preloading the tail tables early).
  - The PE clock p-states ramp only under continuous execution, so
    dummy matmuls fill known PE idle gaps to keep it at high clock.
  - The residual add runs on the PE (identity matmul + rank-1 v
    accumulate into PSUM); the LN normalize reads PSUM directly.

Host-side prep is limited to sharding/layout/dtype (transposes,
fp8/bf16 casts with power-of-2 quantization scales, tile interleave).
"""

import numpy as np
import ml_dtypes
from contextlib import ExitStack

try:
    import concourse.bass as bass
except ImportError:  # pragma: no cover - fallback for bare containers
    import sys
    sys.path.insert(0, "/opt/trn_rl_repo")
    import concourse.bass as bass

import concourse.tile as tile
from concourse import bacc, mybir
from concourse import bass_utils
from concourse.alu_op_type import AluOpType

F32 = mybir.dt.float32
BF16 = mybir.dt.bfloat16
FP8 = mybir.dt.float8e4
I32 = mybir.dt.int32
AF = mybir.ActivationFunctionType
AX = mybir.AxisListType
DR = mybir.MatmulPerfMode.DoubleRow

BS, SEQ, DIM, NH, DH, NNB = 8, 1024, 1024, 16, 64, 50
NT = SEQ // 128   # seq tiles
NJ = DIM // 128   # dim chunks
NU = NJ // 2      # DoubleRow chunk pairs
LN_EPS = 1e-12
N_CORES = 8

# fp8 quantization scales (powers of two; descale folded into copies)
S_W = 16.0     # host premultiplies qw/kw/vw/ow
S_Q = 64.0     # qvec8 = S_Q * q_true
S_QK = 256.0   # qk8 = S_QK * qk_true
S_PN = 32.0    # pn8 = S_PN * pooled_true
S_CX = 32.0    # cxt8 = S_CX * ctx_true
S_BV = 64.0    # bvt8 = S_BV * bvec

_cache = {}


def _build(flags):
    use_qb, use_kb, use_vb, use_ob, use_g, use_b, use_mask = flags
    nc = bacc.Bacc("TRN2", target_bir_lowering=False, debug=False,
                   enable_asserts=True, num_devices=N_CORES)

    def din(name, shape, dt):
        return nc.dram_tensor(name, shape, dt, kind="ExternalInput").ap()

    # big inputs are host-laid-out SBUF images: one DMA each, 8KB rows
    qw8_d = din("qw8", [128, NU * 2 * DIM], FP8)
    kw8_d = din("kw8", [128, NU * 2 * DIM], FP8)
    xt8_d = din("xt8", [128, NU * 2 * SEQ], FP8)
    xbn_d = din("xbn", [128, NT * DIM], BF16)
    vw8_d = din("vw8", [128, NU * 2 * DIM], FP8)
    ow8_d = din("ow8", [128, NU * 2 * DIM], FP8)
    xnb_d = din("xnb", [NNB, DIM], BF16)
    nmp_d = din("nmp", [NNB], BF16)
    nmr_d = din("nmr", [NNB], F32)
    ident_d = din("ident", [16, 48], F32)  # [16,16] identity x3 dtypes packed
    i128b_d = din("i128b", [128, 128], BF16)
    qb_d = din("qb", [DIM], F32) if use_qb else None
    kbt_d = din("kbt", [128, NJ], BF16) if use_kb else None
    vbt_d = din("vbt", [128, NJ], F32) if use_vb else None
    ob_d = din("ob", [DIM], F32) if use_ob else None
    g_d = din("lng", [DIM], F32) if use_g else None
    b_d = din("lnb", [DIM], F32) if use_b else None
    mask_d = din("mask", [SEQ], I32) if use_mask else None
    out_d = nc.dram_tensor("out", [SEQ, DIM], BF16, kind="ExternalOutput").ap()

    with tile.TileContext(nc) as tc, ExitStack() as ctx:
        wpool = ctx.enter_context(tc.tile_pool(name="wts", bufs=1))
        spool = ctx.enter_context(tc.tile_pool(name="small", bufs=1))
        jpool = ctx.enter_context(tc.tile_pool(name="junk", bufs=4))
        opool = ctx.enter_context(tc.tile_pool(name="o", bufs=4))
        pwide = ctx.enter_context(tc.tile_pool(name="pw", bufs=2, space="PSUM"))
        psmall = ctx.enter_context(tc.tile_pool(name="ps", bufs=3, space="PSUM"))
        pdum = ctx.enter_context(tc.tile_pool(name="pd", bufs=1, space="PSUM"))

        # ---------------- DMAs (issue order ~ arrival order) ----------------
        # big tensors needed first go out first; kw8/xt8 ride the ACT queue
        # so two HWDGE queues fill in parallel.
        qw8_t = wpool.tile([128, NU, 2, DIM], FP8, tag="qw8")
        nc.sync.dma_start(qw8_t[:], qw8_d[:])
        kw8_t = wpool.tile([128, NU, 2, DIM], FP8, tag="kw8")
        nc.scalar.dma_start(kw8_t[:], kw8_d[:])
        xnb_t = spool.tile([NNB, DIM], BF16, tag="xnb")
        nc.sync.dma_start(xnb_t[:], xnb_d[:])
        nmp_t = spool.tile([NNB, 1], BF16, tag="nmp")
        nc.sync.dma_start(nmp_t[:], nmp_d.unsqueeze(1))
        nmr_t = spool.tile([1, NNB], F32, tag="nmr")
        nc.sync.dma_start(nmr_t[:], nmr_d.unsqueeze(0))
        id_t = spool.tile([16, 48], F32, tag="ident")
        nc.sync.dma_start(id_t[:], ident_d[:])
        i128b_t = spool.tile([128, 128], BF16, tag="i128b")
        nc.sync.dma_start(i128b_t[:], i128b_d[:])
        xt8_t = wpool.tile([128, NU, 2, SEQ], FP8, tag="xt8")
        nc.scalar.dma_start(xt8_t[:], xt8_d[:])

        def row_tile(d_ap, tag):
            t = spool.tile([1, DIM], F32, tag=tag)
            nc.sync.dma_start(t[:], d_ap.unsqueeze(0))
            return t

        qb_t = row_tile(qb_d, "qbr") if use_qb else None
        ob_t = row_tile(ob_d, "obr") if use_ob else None
        if use_vb:
            vbt_t = spool.tile([128, NJ], F32, tag="vbt")
            nc.sync.dma_start(vbt_t[:], vbt_d[:])
        g_t = row_tile(g_d, "gr") if use_g else None
        b_t = row_tile(b_d, "br") if use_b else None
        if use_kb:
            kbt_t = spool.tile([128, NJ], BF16, tag="kbt")
            nc.sync.dma_start(kbt_t[:], kbt_d[:])
        if use_mask:
            mrow_t = spool.tile([1, SEQ], I32, tag="mrow")
            nc.sync.dma_start(mrow_t[:], mask_d.unsqueeze(0))

        # xbn in 4 chunks so Sum(x^2) passes can chase the DMA
        xbn_t = wpool.tile([128, NT, DIM], BF16, tag="xbn")
        for h in range(4):
            nc.sync.dma_start(xbn_t[:, 2 * h:2 * h + 2, :],
                              xbn_d[:, 2 * h * DIM:(2 * h + 2) * DIM]
                              .rearrange("p (t d) -> p t d", t=2))
        vw8_t = wpool.tile([128, NU, 2, DIM], FP8, tag="vw8")
        nc.sync.dma_start(vw8_t[:], vw8_d[:])
        ow8_t = wpool.tile([128, NU, 2, DIM], FP8, tag="ow8")
        nc.sync.dma_start(ow8_t[:], ow8_d[:])

        i16f_t = id_t[:, 0:16]                     # f32 identity
        i16b_t = spool.tile([16, 16], BF16, tag="i16b")
        nc.vector.tensor_copy(i16b_t[:], id_t[:, 16:32])
        ones11b = spool.tile([1, 1], BF16, tag="ones11b")
        nc.vector.memset(ones11b[:], 1.0)
        ones8_t = spool.tile([128, 2, 4, 4], FP8, tag="ones8")
        nc.vector.memset(ones8_t[:], 1.0)
        ones1x128 = spool.tile([1, 128], F32, tag="ones1x128")
        nc.vector.memset(ones1x128[:], 1.0)
        ones1x128b = spool.tile([1, 128], BF16, tag="ones1x128b")
        nc.vector.memset(ones1x128b[:], 1.0)

        # ACT function-table discipline: 2 resident tables.  Warm up with
        # Exp LAST so the softmax hits a warm table; the tail tables
        # (Sqrt, Identity) are preloaded via dummies mid-kernel.
        dummy_t = spool.tile([1, 1], F32, tag="dummy")
        nc.vector.memset(dummy_t[:], 1.0)
        for fn in (AF.Sqrt, AF.Identity, AF.Square, AF.Copy, AF.Exp):
            nc.scalar.activation(dummy_t[:], dummy_t[:], fn)

        def pe_warm(n):
            # dummy matmuls keep the PE clock ramped through dependency gaps
            for _ in range(n):
                pd = pdum.tile([128, 512], F32, tag="pd")
                nc.tensor.matmul(pd[0:4, :], lhsT=ones8_t[:, :, 0, :],
                                 rhs=qw8_t[:, 0, :, 0:512],
                                 start=True, stop=True, perf_mode=DR)

        def bcast_row(row_ap, out_tile, nrows):
            """out[p, :] = row[0, :] for p in range(nrows), via PE rank-1."""
            n = out_tile.shape[-1]
            pb = pwide.tile([128, DIM], F32, tag="wide")
            for h0 in range(0, n, 512):
                hi = min(h0 + 512, n)
                nc.tensor.matmul(pb[:nrows, h0:hi], lhsT=ones1x128[0:1, 0:nrows],
                                 rhs=row_ap[0:1, h0:hi], start=True, stop=True)
            nc.scalar.copy(out_tile[:nrows, :], pb[:nrows, 0:n])

        # ---------------- neighbor pooling: sxnt fp8 [128, 2, NU, 4] -------
        # sxnt[p, i, u, 0] = sum_n xnb[n, (2u+i)*128+p] * nm[n]  (= cnt * xn)
        # (DoubleRow lhsT needs M in {4, 8, 16} and a non-collapsible AP
        #  whose pair-dim stride is a multiple of 16B, hence the padded
        #  col-4 grouped layouts used for every small DR lhsT below.)
        sxnt_t = spool.tile([128, 2, NU, 4], FP8, tag="sxnt")
        nc.vector.memset(sxnt_t[:], 0.0)
        for c in range(NJ):
            ps = psmall.tile([128, 32], F32, tag="psm")
            nc.tensor.matmul(ps[:, 0:1], lhsT=xnb_t[:, c * 128:(c + 1) * 128],
                             rhs=nmp_t[:], start=True, stop=True)
            nc.vector.tensor_copy(sxnt_t[:, c % 2, c // 2, 0:1], ps[:, 0:1])
        cnt_t = spool.tile([1, 1], F32, tag="cnt")
        nc.vector.reduce_sum(cnt_t[:], nmr_t[:], AX.X)
        rcnt_t = spool.tile([1, 1], F32, tag="rcnt")
        nc.vector.reciprocal(rcnt_t[:], cnt_t[:])

        # ---------------- qvec = S_Q * (qw @ xn + qb) / 8  (bf16 [1, DIM]) -
        # psum = cnt * S_W * (xn @ qw^T);  qvec = psum * rcnt * S_Q/(8*S_W)
        pqv = pwide.tile([128, DIM], F32, tag="wide")
        for h0 in (0, 512):
            for u in range(NU):
                nc.tensor.matmul(pqv[0:4, h0:h0 + 512],
                                 lhsT=sxnt_t[:, :, u, :],
                                 rhs=qw8_t[:, u, :, h0:h0 + 512],
                                 start=(u == 0), stop=(u == NU - 1),
                                 perf_mode=DR)
        qvec_t = spool.tile([1, DIM], BF16, tag="qvec")
        if use_qb:
            qvf_t = spool.tile([1, DIM], F32, tag="qvf")
            nc.vector.tensor_scalar(qvf_t[:], pqv[0:1, :], rcnt_t[:],
                                    S_Q / (8.0 * S_W), AluOpType.mult,
                                    AluOpType.mult)
            qb8_t = spool.tile([1, DIM], F32, tag="qb8")
            nc.vector.tensor_scalar_mul(qb8_t[:], qb_t[:], S_Q / 8.0)
            nc.vector.tensor_tensor(qvf_t[:], qvf_t[:], qb8_t[:],
                                    op=AluOpType.add)
            nc.vector.tensor_copy(qvec_t[:], qvf_t[:])
        else:
            nc.vector.tensor_scalar(qvec_t[:], pqv[0:1, :], rcnt_t[:],
                                    S_Q / (8.0 * S_W), AluOpType.mult,
                                    AluOpType.mult)
        pe_warm(3)

        # ---------------- blk: head-blocked qvec8 (fp8 [128, 2, NU, 16]) ---
        blk_t = spool.tile([128, 2, NU, NH], FP8, tag="blk")
        nc.vector.memset(blk_t[:], 0.0)
        for c in range(NJ):
            pt = psmall.tile([128, 32], BF16, tag="psm")
            nc.tensor.transpose(pt[:, 0:1], qvec_t[0:1, c * 128:(c + 1) * 128],
                                ones11b[:])
            u, i = c // 2, c % 2
            nc.vector.tensor_copy(blk_t[0:64, i, u, 2 * c:2 * c + 1],
                                  pt[0:64, 0:1])
            nc.vector.tensor_copy(blk_t[64:128, i, u, 2 * c + 1:2 * c + 2],
                                  pt[64:128, 0:1])

        # ---------------- qk[h, c] = S_QK * sum_d q[h, d] kw[64h+d, c] -----
        pqk = pwide.tile([128, DIM], F32, tag="wide")
        for h0 in (0, 512):
            for u in range(NU):
                nc.tensor.matmul(pqk[0:NH, h0:h0 + 512],
                                 lhsT=blk_t[:, :, u, :],
                                 rhs=kw8_t[:, u, :, h0:h0 + 512],
                                 start=(u == 0), stop=(u == NU - 1),
                                 perf_mode=DR)
        pe_warm(2)
        qk_t = spool.tile([NH, DIM], BF16, tag="qk")
        nc.vector.tensor_scalar_mul(qk_t[:], pqk[0:NH, :], S_QK / (S_Q * S_W))
        if use_kb:
            # score bias per head: qkb[h] = sum_d q[h,d] kb[64h+d] (true scale)
            blkb_t = spool.tile([128, 2, NU, NH], BF16, tag="blkb")
            nc.vector.tensor_copy(blkb_t[:], blk_t[:])
            pqkb = psmall.tile([128, 32], F32, tag="psm")
            for c in range(NJ):
                u, i = c // 2, c % 2
                nc.tensor.matmul(pqkb[0:NH, 0:1], lhsT=blkb_t[:, i, u, :],
                                 rhs=kbt_t[:, c:c + 1],
                                 start=(c == 0), stop=(c == NJ - 1))
            qkb_t = spool.tile([NH, 1], F32, tag="qkb")
            nc.vector.tensor_scalar_mul(qkb_t[:], pqkb[0:NH, 0:1], 1.0 / S_Q)

        # ---------------- scoresT: psc = S_QK * scores [NH, SEQ] -----------
        qkt_t = spool.tile([128, 2, NU, NH], FP8, tag="qkt")
        for u in range(NU):
            pt = psmall.tile([128, 32], BF16, tag="psm")
            for i in range(2):
                c = 2 * u + i
                nc.tensor.transpose(pt[:, i * NH:(i + 1) * NH],
                                    qk_t[:, c * 128:(c + 1) * 128],
                                    i16b_t[:])
            nc.vector.tensor_copy(qkt_t[:, :, u, :],
                                  pt[:].rearrange("p (i h) -> p i h", i=2))
        psc = pwide.tile([128, DIM], F32, tag="wide")
        for h0 in (0, 512):
            for u in range(NU):
                nc.tensor.matmul(psc[0:NH, h0:h0 + 512],
                                 lhsT=qkt_t[:, :, u, :],
                                 rhs=xt8_t[:, u, :, h0:h0 + 512],
                                 start=(u == 0), stop=(u == NU - 1),
                                 perf_mode=DR)

        # ---- row sums of x via PE (for LN mean), rsc[p, t] = sum_d x[s, d]
        prs = pwide.tile([128, DIM], F32, tag="wide")
        for h0 in (0, 512):
            for u in range(NU):
                nc.tensor.matmul(prs[0:4, h0:h0 + 512],
                                 lhsT=ones8_t[:, :, 0, :],
                                 rhs=xt8_t[:, u, :, h0:h0 + 512],
                                 start=(u == 0), stop=(u == NU - 1),
                                 perf_mode=DR)
        rsr_t = spool.tile([1, SEQ], BF16, tag="rsr")
        nc.vector.tensor_copy(rsr_t[:], prs[0:1, :])
        rsc_t = spool.tile([128, NT], F32, tag="rsc")
        for t in range(NT):
            pt = psmall.tile([128, 32], BF16, tag="psm")
            nc.tensor.transpose(pt[:, 0:1], rsr_t[0:1, t * 128:(t + 1) * 128],
                                ones11b[:])
            nc.vector.tensor_copy(rsc_t[:, t:t + 1], pt[:, 0:1])

        # ---------------- softmax over seq (keys): w bf16 [NH, SEQ] --------
        # scores are O(1) (q is a pooled mean), so exp without
        # max-subtraction is safe; masked keys multiply to exactly 0.
        w_t = spool.tile([NH, SEQ], BF16, tag="w")
        den_t = spool.tile([NH, 1], F32, tag="den")
        expbias = qkb_t[:] if use_kb else 0.0
        if not use_mask:
            nc.scalar.activation(w_t[:], psc[0:NH, :], AF.Exp, bias=expbias,
                                 scale=1.0 / S_QK, accum_out=den_t[:])
        else:
            nc.scalar.activation(w_t[:], psc[0:NH, :], AF.Exp, bias=expbias,
                                 scale=1.0 / S_QK)
            mrowf_t = spool.tile([1, SEQ], F32, tag="mrowf")
            nc.vector.tensor_copy(mrowf_t[:], mrow_t[:])
            ind_t = spool.tile([1, SEQ], F32, tag="ind")
            nc.vector.tensor_scalar(ind_t[:], mrowf_t[:], 0.0, None,
                                    AluOpType.not_equal)
            m16_t = spool.tile([NH, SEQ], F32, tag="m16")
            bcast_row(ind_t, m16_t, NH)
            nc.vector.scalar_tensor_tensor(w_t[:], w_t[:], 1.0, m16_t[:],
                                           AluOpType.mult, AluOpType.mult,
                                           accum_out=den_t[:])
        rden_t = spool.tile([NH, 1], F32, tag="rden")
        nc.vector.reciprocal(rden_t[:], den_t[:])
        rdens_t = spool.tile([NH, 1], F32, tag="rdens")
        nc.vector.tensor_scalar_mul(rdens_t[:], rden_t[:], S_PN)

        # preload the tail ACT tables (Sqrt, Identity) while ACT is idle
        nc.scalar.activation(dummy_t[:], dummy_t[:], AF.Sqrt)
        nc.scalar.activation(dummy_t[:], dummy_t[:], AF.Identity)

        # ---------------- pooled: ppl = sum_s w[h, s] x[s, c]  (bf16) ------
        wt_t = []
        for c in range(NT):
            t = spool.tile([128, NH], BF16, tag=f"wt{c}")
            pt = psmall.tile([128, 32], BF16, tag="psm")
            nc.tensor.transpose(pt[:, 0:NH], w_t[:, c * 128:(c + 1) * 128],
                                i16b_t[:])
            nc.vector.tensor_copy(t[:], pt[:, 0:NH])
            wt_t.append(t)
        ppl = pwide.tile([128, DIM], F32, tag="wide")
        for h0 in (0, 512):
            for c in range(NT):
                nc.tensor.matmul(ppl[0:NH, h0:h0 + 512], lhsT=wt_t[c][:],
                                 rhs=xbn_t[:, c, h0:h0 + 512],
                                 start=(c == 0), stop=(c == NT - 1))

        # ---- Sum(x^2) per row, hidden under the pooled/ctx matmuls -------
        ssq_t = spool.tile([128, NT], F32, tag="ssq")
        for t in range(NT):
            jt = jpool.tile([128, DIM], BF16, tag="junk")
            if t % 2 == 0:
                nc.vector.scalar_tensor_tensor(
                    jt[:], xbn_t[:, t, :], 1.0, xbn_t[:, t, :],
                    AluOpType.mult, AluOpType.mult,
                    accum_out=ssq_t[:, t:t + 1])
            else:
                nc.scalar.activation(jt[:], xbn_t[:, t, :], AF.Square,
                                     accum_out=ssq_t[:, t:t + 1])

        pn_t = spool.tile([NH, DIM], BF16, tag="pn")
        nc.vector.tensor_scalar_mul(pn_t[:], ppl[0:NH, :], rdens_t[:])

        # ---------------- context: diag blocks of pn @ vw^T ----------------
        pnt_t = spool.tile([128, 2, NU, NH], FP8, tag="pnt")
        for u in range(NU):
            pt = psmall.tile([128, 32], BF16, tag="psm")
            for i in range(2):
                c = 2 * u + i
                nc.tensor.transpose(pt[:, i * NH:(i + 1) * NH],
                                    pn_t[:, c * 128:(c + 1) * 128],
                                    i16b_t[:])
            nc.vector.tensor_copy(pnt_t[:, :, u, :],
                                  pt[:].rearrange("p (i h) -> p i h", i=2))
        pcx = pwide.tile([128, DIM], F32, tag="wide")
        for h0 in (0, 512):
            for u in range(NU):
                nc.tensor.matmul(pcx[0:NH, h0:h0 + 512],
                                 lhsT=pnt_t[:, :, u, :],
                                 rhs=vw8_t[:, u, :, h0:h0 + 512],
                                 start=(u == 0), stop=(u == NU - 1),
                                 perf_mode=DR)
        # ctx[o] = pcx[head(o), o]: copy to SBUF, transpose 128-col slices,
        # then pick the two half-column blocks (32-aligned partition bases).
        pcs_t = spool.tile([NH, DIM], BF16, tag="pcs")
        nc.vector.tensor_scalar_mul(pcs_t[:], pcx[0:NH, :], S_CX / (S_PN * S_W))
        pe_warm(2)
        cxt_t = spool.tile([128, 2, NU, 4], FP8, tag="cxt")
        nc.vector.memset(cxt_t[:], 0.0)
        if use_vb:
            cxf_t = spool.tile([128, NJ], F32, tag="cxf")
        for u in range(NU):
            pt = psmall.tile([128, 32], BF16, tag="psm")
            for i in range(2):
                c = 2 * u + i
                nc.tensor.transpose(pt[:, i * NH:(i + 1) * NH],
                                    pcs_t[:, c * 128:(c + 1) * 128],
                                    i16b_t[:])
            for i in range(2):
                c = 2 * u + i
                dst_hi = cxf_t[0:64, c:c + 1] if use_vb else cxt_t[0:64, i, u, 0:1]
                dst_lo = cxf_t[64:128, c:c + 1] if use_vb else cxt_t[64:128, i, u, 0:1]
                nc.vector.tensor_copy(dst_hi,
                                      pt[0:64, i * NH + 2 * c:i * NH + 2 * c + 1])
                nc.vector.tensor_copy(dst_lo,
                                      pt[64:128, i * NH + 2 * c + 1:i * NH + 2 * c + 2])
        if use_vb:
            # cxf holds S_CX * ctx; add S_CX * vb then quantize
            vbs_t = spool.tile([128, NJ], F32, tag="vbs")
            nc.vector.tensor_scalar_mul(vbs_t[:], vbt_t[:], S_CX)
            nc.vector.tensor_tensor(cxf_t[:], cxf_t[:], vbs_t[:],
                                    op=AluOpType.add)
            for c in range(NJ):
                nc.vector.tensor_copy(cxt_t[:, c % 2, c // 2, 0:1],
                                      cxf_t[:, c:c + 1])

        # ---------------- out_vec = ow @ ctx + ob --------------------------
        pov = pwide.tile([128, DIM], F32, tag="wide")
        for h0 in (0, 512):
            for u in range(NU):
                nc.tensor.matmul(pov[0:4, h0:h0 + 512],
                                 lhsT=cxt_t[:, :, u, :],
                                 rhs=ow8_t[:, u, :, h0:h0 + 512],
                                 start=(u == 0), stop=(u == NU - 1),
                                 perf_mode=DR)
        pe_warm(2)
        bvec_t = spool.tile([1, DIM], F32, tag="bvec")
        nc.vector.tensor_scalar_mul(bvec_t[:], pov[0:1, :], 1.0 / (S_CX * S_W))
        if use_ob:
            nc.vector.tensor_tensor(bvec_t[:], bvec_t[:], ob_t[:],
                                    op=AluOpType.add)

        # ---------------- residual + LayerNorm -----------------------------
        # h = x + v (v = bvec broadcast over rows).  Per row s:
        #   mu_h[s]  = mu_x[s] + mu_v
        #   var_h[s] = Sxx[s]/D - mu_x^2 + var_v + 2*(Sxv[s]/D - mu_x*mu_v)
        # mu_x comes from the early PE row sums; Sxx from the early square
        # passes; only the cheap cross-term Sxv = x @ v (PE fp8 gemv) and
        # batched [128, NT] fixups happen after bvec is known.  The h add
        # itself runs on the PE (identity matmul + rank-1 accumulate into
        # PSUM); the normalize reads PSUM directly.
        bvr16_t = spool.tile([1, DIM], BF16, tag="bvr16")
        nc.vector.tensor_copy(bvr16_t[:], bvec_t[:])
        if use_g:
            gb_t = spool.tile([128, DIM], F32, tag="gb")
            bcast_row(g_t, gb_t, 128)
        if use_b:
            bb_t = spool.tile([128, DIM], F32, tag="bb")
            bcast_row(b_t, bb_t, 128)

        # scalars of v: sv = [mu_v, var_v] -> bsc [128, 2] via PE rank-1
        sv_t = spool.tile([1, 2], F32, tag="sv")
        nc.vector.reduce_sum(sv_t[0:1, 0:1], bvec_t[:], AX.X)
        junk1_t = spool.tile([1, DIM], F32, tag="junk1")
        nc.vector.scalar_tensor_tensor(junk1_t[:], bvec_t[:], 1.0, bvec_t[:],
                                       AluOpType.mult, AluOpType.mult,
                                       accum_out=sv_t[0:1, 1:2])
        nc.vector.tensor_scalar_mul(sv_t[:], sv_t[:], 1.0 / DIM)
        muv2_t = spool.tile([1, 1], F32, tag="muv2")
        nc.vector.tensor_tensor(muv2_t[:], sv_t[0:1, 0:1], sv_t[0:1, 0:1],
                                op=AluOpType.mult)
        nc.vector.tensor_tensor(sv_t[0:1, 1:2], sv_t[0:1, 1:2], muv2_t[:],
                                op=AluOpType.subtract)
        pbs = psmall.tile([128, 32], F32, tag="psm")
        nc.tensor.matmul(pbs[:, 0:2], lhsT=ones1x128[:], rhs=sv_t[:],
                         start=True, stop=True)
        bsc_t = spool.tile([128, 2], F32, tag="bsc")
        nc.vector.tensor_copy(bsc_t[:], pbs[:, 0:2])

        # Sxv row via PE: bvec fp8 chunks against xt8
        bvrs_t = spool.tile([1, DIM], BF16, tag="bvrs")
        nc.vector.tensor_scalar_mul(bvrs_t[:], bvec_t[:], S_BV)
        bvt_t = spool.tile([128, 2, NU, 4], FP8, tag="bvt")
        nc.vector.memset(bvt_t[:], 0.0)
        for c in range(NJ):
            pt = psmall.tile([128, 32], BF16, tag="psm")
            nc.tensor.transpose(pt[:, 0:1], bvrs_t[0:1, c * 128:(c + 1) * 128],
                                ones11b[:])
            nc.vector.tensor_copy(bvt_t[:, c % 2, c // 2, 0:1], pt[:, 0:1])
        psxv = pwide.tile([128, DIM], F32, tag="wide")
        for h0 in (0, 512):
            for u in range(NU):
                nc.tensor.matmul(psxv[0:4, h0:h0 + 512],
                                 lhsT=bvt_t[:, :, u, :],
                                 rhs=xt8_t[:, u, :, h0:h0 + 512],
                                 start=(u == 0), stop=(u == NU - 1),
                                 perf_mode=DR)
        sxvr_t = spool.tile([1, SEQ], BF16, tag="sxvr")
        nc.vector.tensor_scalar_mul(sxvr_t[:], psxv[0:1, :], 1.0 / S_BV)
        sxvc_t = spool.tile([128, NT], F32, tag="sxvc")
        for t in range(NT):
            pt = psmall.tile([128, 32], BF16, tag="psm")
            nc.tensor.transpose(pt[:, 0:1], sxvr_t[0:1, t * 128:(t + 1) * 128],
                                ones11b[:])
            nc.vector.tensor_copy(sxvc_t[:, t:t + 1], pt[:, 0:1])

        # batched fixups over [128, NT]
        mux_t = spool.tile([128, NT], F32, tag="mux")
        nc.vector.tensor_scalar_mul(mux_t[:], rsc_t[:], 1.0 / DIM)
        m2v_t = spool.tile([128, 1], F32, tag="m2v")
        nc.vector.tensor_scalar_mul(m2v_t[:], bsc_t[:, 0:1], 2.0)
        tmp1_t = spool.tile([128, NT], F32, tag="tmp1")
        nc.vector.tensor_scalar(tmp1_t[:], mux_t[:], m2v_t[:], None,
                                AluOpType.add)
        tmp2_t = spool.tile([128, NT], F32, tag="tmp2")
        nc.vector.tensor_tensor(tmp2_t[:], mux_t[:], tmp1_t[:],
                                op=AluOpType.mult)
        tmp3_t = spool.tile([128, NT], F32, tag="tmp3")
        nc.vector.scalar_tensor_tensor(tmp3_t[:], sxvc_t[:], 2.0, ssq_t[:],
                                       AluOpType.mult, AluOpType.add)
        varh_t = spool.tile([128, NT], F32, tag="varh")
        nc.vector.scalar_tensor_tensor(varh_t[:], tmp3_t[:], 1.0 / DIM,
                                       tmp2_t[:], AluOpType.mult,
                                       AluOpType.subtract)
        nc.vector.tensor_scalar(varh_t[:], varh_t[:], bsc_t[:, 1:2], LN_EPS,
                                AluOpType.add, AluOpType.add)
        rv_t = spool.tile([128, NT], F32, tag="rv")
        nc.vector.reciprocal(rv_t[:], varh_t[:])
        rstd_t = spool.tile([128, NT], F32, tag="rstd")
        nc.scalar.sqrt(rstd_t[:], rv_t[:])
        muh_t = spool.tile([128, NT], F32, tag="muh")
        nc.vector.tensor_scalar(muh_t[:], mux_t[:], bsc_t[:, 0:1], None,
                                AluOpType.add)
        nmrw_t = spool.tile([128, NT], F32, tag="nmrw")
        nc.vector.scalar_tensor_tensor(nmrw_t[:], muh_t[:], -1.0, rstd_t[:],
                                       AluOpType.mult, AluOpType.mult)

        # per tile: PE h-add into PSUM, ACT/DVE normalize, DMA out
        for t in range(NT):
            ph = pwide.tile([128, DIM], F32, tag="wide")
            for h0 in (0, 512):
                nc.tensor.matmul(ph[:, h0:h0 + 512], lhsT=i128b_t[:],
                                 rhs=xbn_t[:, t, h0:h0 + 512],
                                 start=True, stop=False)
                nc.tensor.matmul(ph[:, h0:h0 + 512], lhsT=ones1x128b[:],
                                 rhs=bvr16_t[0:1, h0:h0 + 512],
                                 start=False, stop=True)
            o_t = opool.tile([128, DIM], BF16, tag="o")
            if t % 3 != 2:
                nc.scalar.activation(o_t[:], ph[:], AF.Identity,
                                     bias=nmrw_t[:, t:t + 1],
                                     scale=rstd_t[:, t:t + 1])
            else:
                nc.vector.tensor_scalar(o_t[:], ph[:], rstd_t[:, t:t + 1],
                                        nmrw_t[:, t:t + 1], AluOpType.mult,
                                        AluOpType.add)
            if use_g:
                nc.vector.tensor_tensor(o_t[:], o_t[:], gb_t[:],
                                        op=AluOpType.mult)
            if use_b:
                nc.vector.tensor_tensor(o_t[:], o_t[:], bb_t[:],
                                        op=AluOpType.add)
            nc.sync.dma_start(out_d[t * 128:(t + 1) * 128, :], o_t[:])

    nc.compile()
    return nc


def _get_program(flags):
    if flags not in _cache:
        _cache[flags] = _build(flags)
    return _cache[flags]


def _dr_pack(a):
    """[1024, W] -> [128, 4*2*W] DoubleRow-interleaved SBUF image."""
    w = a.shape[1]
    return np.ascontiguousarray(
        a.reshape(NU, 2, 128, w).transpose(2, 0, 1, 3).reshape(128, NU * 2 * w))


def build_in_maps(inputs):
    f32 = lambda a: np.ascontiguousarray(np.asarray(a, np.float32))
    bf = ml_dtypes.bfloat16
    f8 = ml_dtypes.float8_e4m3
    x = f32(inputs["x"])
    xnb = f32(inputs["x_neighbor"])
    mask = np.ascontiguousarray(np.asarray(inputs["mask"], np.int32))
    nmask = f32(inputs["neighbor_mask"])
    qw, qb = f32(inputs["qw"]), f32(inputs["qb"])
    kw, kb = f32(inputs["kw"]), f32(inputs["kb"])
    vw, vb = f32(inputs["vw"]), f32(inputs["vb"])
    ow, ob = f32(inputs["ow"]), f32(inputs["ob"])
    ln_g, ln_b = f32(inputs["ln_g"]), f32(inputs["ln_b"])

    flags = (bool(qb.any()), bool(kb.any()), bool(vb.any()), bool(ob.any()),
             bool((ln_g != 1.0).any()), bool(ln_b.any()),
             bool((mask == 0).any()))
    use_qb, use_kb, use_vb, use_ob, use_g, use_b, use_mask = flags

    qw8 = _dr_pack((S_W * qw.T).astype(f8))
    kw8 = _dr_pack((S_W * kw).astype(f8))
    vw8 = _dr_pack((S_W * vw.T).astype(f8))
    ow8 = _dr_pack((S_W * ow.T).astype(f8))
    ident = np.zeros((16, 48), np.float32)
    ident[:, 0:16] = np.eye(16)
    ident[:, 16:32] = np.eye(16)
    ident[:, 32:48] = np.eye(16)
    i128b = np.eye(128, dtype=bf)

    in_maps = []
    for b in range(BS):
        m = {
            "qw8": qw8, "kw8": kw8, "vw8": vw8, "ow8": ow8,
            "xt8": _dr_pack(np.ascontiguousarray(x[b].T).astype(f8)),
            "xbn": np.ascontiguousarray(
                x[b].reshape(NT, 128, DIM).transpose(1, 0, 2)
                .reshape(128, NT * DIM)).astype(bf),
            "xnb": np.ascontiguousarray(xnb[b]).astype(bf),
            "nmp": np.ascontiguousarray(nmask[b]).astype(bf),
            "nmr": np.ascontiguousarray(nmask[b]),
            "ident": ident,
            "i128b": i128b,
        }
        if use_qb:
            m["qb"] = qb
        if use_kb:
            m["kbt"] = np.ascontiguousarray(kb.reshape(NJ, 128).T).astype(bf)
        if use_vb:
            m["vbt"] = np.ascontiguousarray(vb.reshape(NJ, 128).T)
        if use_ob:
            m["ob"] = ob
        if use_g:
            m["lng"] = ln_g
        if use_b:
            m["lnb"] = ln_b
        if use_mask:
            m["mask"] = np.ascontiguousarray(mask[b])
        in_maps.append(m)
    return flags, in_maps


def kernel(**inputs):
    flags, in_maps = build_in_maps(inputs)
    nc = _get_program(flags)
    res = bass_utils.run_bass_kernel_spmd(nc, in_maps,
                                          core_ids=list(range(N_CORES)))
    return np.stack([res.results[b]["out"] for b in range(BS)]).astype(np.float32)


# revision 11
# speedup vs baseline: 1.1813x; 1.1813x over previous
"""Trainium2 Bass kernel for nn_NeighborAttention.

Key algebraic structure exploited: the attention query is a single
mean-pooled neighbor vector per batch, broadcast over the sequence.
Hence the [seq, seq] attention collapses to one weight vector per head
([nh, seq]) and the whole attention output is a single vector per batch
added to every row of x before the final LayerNorm.  The k/v
projections are never materialized: scores are computed as
x @ (q^T kw) and the value path as (w @ x) @ vw^T, reducing compute
from ~34 GFLOP to ~0.6 GFLOP.  Sharding: data-parallel over batch
(one batch element per NeuronCore, 8 cores).

Precision plan (output gate is 2e-2 relative error; final output is
dominated by LayerNorm(x + small-vector), so the attention path has
~100x error headroom):
  - residual x copy + output: bf16
  - all four weight matrices + x^T (scores/Sxv operand): fp8 e4m3,
    DoubleRow matmuls (2 contraction tiles per instruction)
  - softmax weights: bf16 (values are 1 +- 0.06; fp8 would flush the
    variation), so the pooled matmul runs bf16 against the residual
    x copy
  - fp8 operands carry power-of-2 scales chosen to sit in e4m3's
    normal range; all scales are folded into PSUM->SBUF copies.

Scheduling notes (from HW traces):
  - GpSimd is never used: its per-op ucode library swaps cost ~7us.
  - ACT runs only table-stable work per phase (Exp mid, Sqrt+Identity
    tail, with dummy activations preloading the tail tables early).
  - The PE clock p-states ramp only under continuous execution, so
    dummy matmuls fill known PE idle gaps to keep it at high clock.
  - The residual add runs on the PE (identity matmul + rank-1 v
    accumulate into PSUM); the LN normalize reads PSUM directly.

Host-side prep is limited to sharding/layout/dtype (transposes,
fp8/bf16 casts with power-of-2 quantization scales, tile interleave).
"""

import numpy as np
import ml_dtypes
from contextlib import ExitStack

try:
    import concourse.bass as bass
except ImportError:  # pragma: no cover - fallback for bare containers
    import sys
    sys.path.insert(0, "/opt/trn_rl_repo")
    import concourse.bass as bass

import concourse.tile as tile
from concourse import bacc, mybir
from concourse import bass_utils
from concourse.alu_op_type import AluOpType

F32 = mybir.dt.float32
BF16 = mybir.dt.bfloat16
FP8 = mybir.dt.float8e4
I32 = mybir.dt.int32
AF = mybir.ActivationFunctionType
AX = mybir.AxisListType
DR = mybir.MatmulPerfMode.DoubleRow

BS, SEQ, DIM, NH, DH, NNB = 8, 1024, 1024, 16, 64, 50
NT = SEQ // 128   # seq tiles
NJ = DIM // 128   # dim chunks
NU = NJ // 2      # DoubleRow chunk pairs
LN_EPS = 1e-12
N_CORES = 8

# fp8 quantization scales (powers of two; descale folded into copies)
S_W = 16.0     # host premultiplies qw/kw/vw/ow
S_Q = 64.0     # qvec8 = S_Q * q_true
S_QK = 256.0   # qk8 = S_QK * qk_true
S_PN = 32.0    # pn8 = S_PN * pooled_true
S_CX = 32.0    # cxt8 = S_CX * ctx_true
S_BV = 64.0    # bvt8 = S_BV * bvec

_cache = {}


def _build(flags):
    use_qb, use_kb, use_vb, use_ob, use_g, use_b, use_mask = flags
    nc = bacc.Bacc("TRN2", target_bir_lowering=False, debug=False,
                   enable_asserts=True, num_devices=N_CORES)

    def din(name, shape, dt):
        return nc.dram_tensor(name, shape, dt, kind="ExternalInput").ap()

    # big inputs are host-laid-out SBUF images: one DMA each, 8KB rows
    qw8_d = din("qw8", [128, NU * 2 * DIM], FP8)
    kw8_d = din("kw8", [128, NU * 2 * DIM], FP8)
    xt8_d = din("xt8", [128, NU * 2 * SEQ], FP8)
    xbn_d = din("xbn", [128, NT * DIM], BF16)
    vw8_d = din("vw8", [128, NU * 2 * DIM], FP8)
    ow8_d = din("ow8", [128, NU * 2 * DIM], FP8)
    xnb_d = din("xnb", [NNB, DIM], BF16)
    nmp_d = din("nmp", [NNB], BF16)
    nmr_d = din("nmr", [NNB], F32)
    ident_d = din("ident", [16, 48], F32)  # [16,16] identity x3 dtypes packed
    i128b_d = din("i128b", [128, 128], BF16)
    qb_d = din("qb", [DIM], F32) if use_qb else None
    kbt_d = din("kbt", [128, NJ], BF16) if use_kb else None
    vbt_d = din("vbt", [128, NJ], F32) if use_vb else None
    ob_d = din("ob", [DIM], F32) if use_ob else None
    g_d = din("lng", [DIM], F32) if use_g else None
    b_d = din("lnb", [DIM], F32) if use_b else None
    mask_d = din("mask", [SEQ], I32) if use_mask else None
    out_d = nc.dram_tensor("out", [SEQ, DIM], BF16, kind="ExternalOutput").ap()

    with tile.TileContext(nc) as tc, ExitStack() as ctx:
        wpool = ctx.enter_context(tc.tile_pool(name="wts", bufs=1))
        spool = ctx.enter_context(tc.tile_pool(name="small", bufs=1))
        jpool = ctx.enter_context(tc.tile_pool(name="junk", bufs=4))
        opool = ctx.enter_context(tc.tile_pool(name="o", bufs=4))
        pwide = ctx.enter_context(tc.tile_pool(name="pw", bufs=2, space="PSUM"))
        psmall = ctx.enter_context(tc.tile_pool(name="ps", bufs=3, space="PSUM"))
        pdum = ctx.enter_context(tc.tile_pool(name="pd", bufs=1, space="PSUM"))

        # ---------------- DMAs (issue order ~ arrival order) ----------------
        # big tensors needed first go out first; kw8/xt8 ride the ACT queue
        # so two HWDGE queues fill in parallel.
        qw8_t = wpool.tile([128, NU, 2, DIM], FP8, tag="qw8")
        nc.sync.dma_start(qw8_t[:], qw8_d[:])
        kw8_t = wpool.tile([128, NU, 2, DIM], FP8, tag="kw8")
        nc.scalar.dma_start(kw8_t[:], kw8_d[:])
        xnb_t = spool.tile([NNB, DIM], BF16, tag="xnb")
        nc.sync.dma_start(xnb_t[:], xnb_d[:])
        nmp_t = spool.tile([NNB, 1], BF16, tag="nmp")
        nc.sync.dma_start(nmp_t[:], nmp_d.unsqueeze(1))
        nmr_t = spool.tile([1, NNB], F32, tag="nmr")
        nc.sync.dma_start(nmr_t[:], nmr_d.unsqueeze(0))
        id_t = spool.tile([16, 48], F32, tag="ident")
        nc.sync.dma_start(id_t[:], ident_d[:])
        i128b_t = spool.tile([128, 128], BF16, tag="i128b")
        nc.sync.dma_start(i128b_t[:], i128b_d[:])
        xt8_t = wpool.tile([128, NU, 2, SEQ], FP8, tag="xt8")
        nc.scalar.dma_start(xt8_t[:], xt8_d[:])

        def row_tile(d_ap, tag):
            t = spool.tile([1, DIM], F32, tag=tag)
            nc.sync.dma_start(t[:], d_ap.unsqueeze(0))
            return t

        qb_t = row_tile(qb_d, "qbr") if use_qb else None
        ob_t = row_tile(ob_d, "obr") if use_ob else None
        if use_vb:
            vbt_t = spool.tile([128, NJ], F32, tag="vbt")
            nc.sync.dma_start(vbt_t[:], vbt_d[:])
        g_t = row_tile(g_d, "gr") if use_g else None
        b_t = row_tile(b_d, "br") if use_b else None
        if use_kb:
            kbt_t = spool.tile([128, NJ], BF16, tag="kbt")
            nc.sync.dma_start(kbt_t[:], kbt_d[:])
        if use_mask:
            mrow_t = spool.tile([1, SEQ], I32, tag="mrow")
            nc.sync.dma_start(mrow_t[:], mask_d.unsqueeze(0))

        # xbn in 4 chunks so Sum(x^2) passes can chase the DMA
        xbn_t = wpool.tile([128, NT, DIM], BF16, tag="xbn")
        for h in range(4):
            nc.sync.dma_start(xbn_t[:, 2 * h:2 * h + 2, :],
                              xbn_d[:, 2 * h * DIM:(2 * h + 2) * DIM]
                              .rearrange("p (t d) -> p t d", t=2))
        vw8_t = wpool.tile([128, NU, 2, DIM], FP8, tag="vw8")
        nc.sync.dma_start(vw8_t[:], vw8_d[:])
        ow8_t = wpool.tile([128, NU, 2, DIM], FP8, tag="ow8")
        nc.sync.dma_start(ow8_t[:], ow8_d[:])

        i16f_t = id_t[:, 0:16]                     # f32 identity
        i16b_t = spool.tile([16, 16], BF16, tag="i16b")
        nc.vector.tensor_copy(i16b_t[:], id_t[:, 16:32])
        ones11b = spool.tile([1, 1], BF16, tag="ones11b")
        nc.vector.memset(ones11b[:], 1.0)
        ones8_t = spool.tile([128, 2, 4, 4], FP8, tag="ones8")
        nc.vector.memset(ones8_t[:], 1.0)
        ones1x128 = spool.tile([1, 128], F32, tag="ones1x128")
        nc.vector.memset(ones1x128[:], 1.0)
        ones1x128b = spool.tile([1, 128], BF16, tag="ones1x128b")
        nc.vector.memset(ones1x128b[:], 1.0)

        # ACT function-table discipline: 2 resident tables.  Warm up with
        # Exp LAST so the softmax hits a warm table; the tail tables
        # (Sqrt, Identity) are preloaded via dummies mid-kernel.
        dummy_t = spool.tile([1, 1], F32, tag="dummy")
        nc.vector.memset(dummy_t[:], 1.0)
        for fn in (AF.Square, AF.Copy, AF.Sqrt, AF.Identity, AF.Exp):
            nc.scalar.activation(dummy_t[:], dummy_t[:], fn)

        def pe_warm(n):
            # dummy matmuls keep the PE clock ramped through dependency gaps
            for _ in range(n):
                pd = pdum.tile([128, 512], F32, tag="pd")
                nc.tensor.matmul(pd[0:4, :], lhsT=ones8_t[:, :, 0, :],
                                 rhs=qw8_t[:, 0, :, 0:512],
                                 start=True, stop=True, perf_mode=DR)

        def bcast_row(row_ap, out_tile, nrows):
            """out[p, :] = row[0, :] for p in range(nrows), via PE rank-1."""
            n = out_tile.shape[-1]
            pb = pwide.tile([128, DIM], F32, tag="wide")
            for h0 in range(0, n, 512):
                hi = min(h0 + 512, n)
                nc.tensor.matmul(pb[:nrows, h0:hi], lhsT=ones1x128[0:1, 0:nrows],
                                 rhs=row_ap[0:1, h0:hi], start=True, stop=True)
            nc.scalar.copy(out_tile[:nrows, :], pb[:nrows, 0:n])

        # ---------------- neighbor pooling: sxnt fp8 [128, 2, NU, 4] -------
        # sxnt[p, i, u, 0] = sum_n xnb[n, (2u+i)*128+p] * nm[n]  (= cnt * xn)
        # (DoubleRow lhsT needs M in {4, 8, 16} and a non-collapsible AP
        #  whose pair-dim stride is a multiple of 16B, hence the padded
        #  col-4 grouped layouts used for every small DR lhsT below.)
        sxnt_t = spool.tile([128, 2, NU, 4], FP8, tag="sxnt")
        nc.vector.memset(sxnt_t[:], 0.0)
        for c in range(NJ):
            ps = psmall.tile([128, 32], F32, tag="psm")
            nc.tensor.matmul(ps[:, 0:1], lhsT=xnb_t[:, c * 128:(c + 1) * 128],
                             rhs=nmp_t[:], start=True, stop=True)
            nc.vector.tensor_copy(sxnt_t[:, c % 2, c // 2, 0:1], ps[:, 0:1])
        cnt_t = spool.tile([1, 1], F32, tag="cnt")
        nc.vector.reduce_sum(cnt_t[:], nmr_t[:], AX.X)
        rcnt_t = spool.tile([1, 1], F32, tag="rcnt")
        nc.vector.reciprocal(rcnt_t[:], cnt_t[:])

        # ---------------- qvec = S_Q * (qw @ xn + qb) / 8  (bf16 [1, DIM]) -
        # psum = cnt * S_W * (xn @ qw^T);  qvec = psum * rcnt * S_Q/(8*S_W)
        pqv = pwide.tile([128, DIM], F32, tag="wide")
        for h0 in (0, 512):
            for u in range(NU):
                nc.tensor.matmul(pqv[0:4, h0:h0 + 512],
                                 lhsT=sxnt_t[:, :, u, :],
                                 rhs=qw8_t[:, u, :, h0:h0 + 512],
                                 start=(u == 0), stop=(u == NU - 1),
                                 perf_mode=DR)
        qvec_t = spool.tile([1, DIM], BF16, tag="qvec")
        if use_qb:
            qvf_t = spool.tile([1, DIM], F32, tag="qvf")
            nc.vector.tensor_scalar(qvf_t[:], pqv[0:1, :], rcnt_t[:],
                                    S_Q / (8.0 * S_W), AluOpType.mult,
                                    AluOpType.mult)
            qb8_t = spool.tile([1, DIM], F32, tag="qb8")
            nc.vector.tensor_scalar_mul(qb8_t[:], qb_t[:], S_Q / 8.0)
            nc.vector.tensor_tensor(qvf_t[:], qvf_t[:], qb8_t[:],
                                    op=AluOpType.add)
            nc.vector.tensor_copy(qvec_t[:], qvf_t[:])
        else:
            nc.vector.tensor_scalar(qvec_t[:], pqv[0:1, :], rcnt_t[:],
                                    S_Q / (8.0 * S_W), AluOpType.mult,
                                    AluOpType.mult)
        pe_warm(3)

        # ---------------- blk: head-blocked qvec8 (fp8 [128, 2, NU, 16]) ---
        blk_t = spool.tile([128, 2, NU, NH], FP8, tag="blk")
        nc.vector.memset(blk_t[:], 0.0)
        for c in range(NJ):
            pt = psmall.tile([128, 32], BF16, tag="psm")
            nc.tensor.transpose(pt[:, 0:1], qvec_t[0:1, c * 128:(c + 1) * 128],
                                ones11b[:])
            u, i = c // 2, c % 2
            nc.vector.tensor_copy(blk_t[0:64, i, u, 2 * c:2 * c + 1],
                                  pt[0:64, 0:1])
            nc.vector.tensor_copy(blk_t[64:128, i, u, 2 * c + 1:2 * c + 2],
                                  pt[64:128, 0:1])

        # ---------------- qk[h, c] = S_QK * sum_d q[h, d] kw[64h+d, c] -----
        pqk = pwide.tile([128, DIM], F32, tag="wide")
        for h0 in (0, 512):
            for u in range(NU):
                nc.tensor.matmul(pqk[0:NH, h0:h0 + 512],
                                 lhsT=blk_t[:, :, u, :],
                                 rhs=kw8_t[:, u, :, h0:h0 + 512],
                                 start=(u == 0), stop=(u == NU - 1),
                                 perf_mode=DR)
        pe_warm(2)
        qk_t = spool.tile([NH, DIM], BF16, tag="qk")
        nc.scalar.activation(qk_t[:], pqk[0:NH, :], AF.Identity,
                             scale=S_QK / (S_Q * S_W))
        if use_kb:
            # score bias per head: qkb[h] = sum_d q[h,d] kb[64h+d] (true scale)
            blkb_t = spool.tile([128, 2, NU, NH], BF16, tag="blkb")
            nc.vector.tensor_copy(blkb_t[:], blk_t[:])
            pqkb = psmall.tile([128, 32], F32, tag="psm")
            for c in range(NJ):
                u, i = c // 2, c % 2
                nc.tensor.matmul(pqkb[0:NH, 0:1], lhsT=blkb_t[:, i, u, :],
                                 rhs=kbt_t[:, c:c + 1],
                                 start=(c == 0), stop=(c == NJ - 1))
            qkb_t = spool.tile([NH, 1], F32, tag="qkb")
            nc.vector.tensor_scalar_mul(qkb_t[:], pqkb[0:NH, 0:1], 1.0 / S_Q)

        # ---------------- scoresT: psc = S_QK * scores [NH, SEQ] -----------
        # lhsT is M=40: cols 0-15 carry qk^T, col 32 carries ones so psum
        # row 32 (PSUM reads need 32-aligned partition bases) comes out as
        # the per-seq-row sum of x (for the LN mean)
        qkt_t = spool.tile([128, 2, NU, 40], FP8, tag="qkt")
        nc.vector.memset(qkt_t[:], 0.0)
        nc.vector.memset(qkt_t[:, :, :, 32:33], 1.0)
        for u in range(NU):
            pt = psmall.tile([128, 32], BF16, tag="psm")
            for i in range(2):
                c = 2 * u + i
                nc.tensor.transpose(pt[:, i * NH:(i + 1) * NH],
                                    qk_t[:, c * 128:(c + 1) * 128],
                                    i16b_t[:])
            nc.vector.tensor_copy(qkt_t[:, :, u, 0:NH],
                                  pt[:].rearrange("p (i h) -> p i h", i=2))
        psc = pwide.tile([128, DIM], F32, tag="wide")
        for h0 in (0, 512):
            for u in range(NU):
                nc.tensor.matmul(psc[0:40, h0:h0 + 512],
                                 lhsT=qkt_t[:, :, u, :],
                                 rhs=xt8_t[:, u, :, h0:h0 + 512],
                                 start=(u == 0), stop=(u == NU - 1),
                                 perf_mode=DR)
        rsr_t = spool.tile([1, SEQ], BF16, tag="rsr")
        nc.vector.tensor_copy(rsr_t[:], psc[32:33, :])
        rsc_t = spool.tile([128, NT], F32, tag="rsc")
        for t in range(NT):
            pt = psmall.tile([128, 32], BF16, tag="psm")
            nc.tensor.transpose(pt[:, 0:1], rsr_t[0:1, t * 128:(t + 1) * 128],
                                ones11b[:])
            nc.vector.tensor_copy(rsc_t[:, t:t + 1], pt[:, 0:1])

        # ---------------- softmax over seq (keys): w bf16 [NH, SEQ] --------
        # scores are O(1) (q is a pooled mean), so exp without
        # max-subtraction is safe; masked keys multiply to exactly 0.
        w_t = spool.tile([NH, SEQ], BF16, tag="w")
        den_t = spool.tile([NH, 1], F32, tag="den")
        expbias = qkb_t[:] if use_kb else 0.0
        if not use_mask:
            nc.scalar.activation(w_t[:], psc[0:NH, :], AF.Exp, bias=expbias,
                                 scale=1.0 / S_QK, accum_out=den_t[:])
        else:
            nc.scalar.activation(w_t[:], psc[0:NH, :], AF.Exp, bias=expbias,
                                 scale=1.0 / S_QK)
            mrowf_t = spool.tile([1, SEQ], F32, tag="mrowf")
            nc.vector.tensor_copy(mrowf_t[:], mrow_t[:])
            ind_t = spool.tile([1, SEQ], F32, tag="ind")
            nc.vector.tensor_scalar(ind_t[:], mrowf_t[:], 0.0, None,
                                    AluOpType.not_equal)
            m16_t = spool.tile([NH, SEQ], F32, tag="m16")
            bcast_row(ind_t, m16_t, NH)
            nc.vector.scalar_tensor_tensor(w_t[:], w_t[:], 1.0, m16_t[:],
                                           AluOpType.mult, AluOpType.mult,
                                           accum_out=den_t[:])
        rden_t = spool.tile([NH, 1], F32, tag="rden")
        nc.vector.reciprocal(rden_t[:], den_t[:])
        rdens_t = spool.tile([NH, 1], F32, tag="rdens")
        nc.vector.tensor_scalar_mul(rdens_t[:], rden_t[:], S_PN)

        # preload the tail Sqrt table while ACT is idle (evicts Exp)
        nc.scalar.activation(dummy_t[:], dummy_t[:], AF.Sqrt)

        # ---------------- pooled: ppl = sum_s w[h, s] x[s, c]  (bf16) ------
        wt_t = []
        for c in range(NT):
            t = spool.tile([128, NH], BF16, tag=f"wt{c}")
            pt = psmall.tile([128, 32], BF16, tag="psm")
            nc.tensor.transpose(pt[:, 0:NH], w_t[:, c * 128:(c + 1) * 128],
                                i16b_t[:])
            nc.vector.tensor_copy(t[:], pt[:, 0:NH])
            wt_t.append(t)
        ppl = pwide.tile([128, DIM], F32, tag="wide")
        for h0 in (0, 512):
            for c in range(NT):
                nc.tensor.matmul(ppl[0:NH, h0:h0 + 512], lhsT=wt_t[c][:],
                                 rhs=xbn_t[:, c, h0:h0 + 512],
                                 start=(c == 0), stop=(c == NT - 1))

        # ---- Sum(x^2) per row, hidden under the pooled/ctx matmuls -------
        ssq_t = spool.tile([128, NT], F32, tag="ssq")
        for t in range(NT):
            jt = jpool.tile([128, DIM], BF16, tag="junk")
            nc.vector.scalar_tensor_tensor(
                jt[:], xbn_t[:, t, :], 1.0, xbn_t[:, t, :],
                AluOpType.mult, AluOpType.mult,
                accum_out=ssq_t[:, t:t + 1])

        pn_t = spool.tile([NH, DIM], BF16, tag="pn")
        nc.scalar.activation(pn_t[:], ppl[0:NH, :], AF.Identity,
                             scale=rdens_t[:])

        # ---------------- context: diag blocks of pn @ vw^T ----------------
        pnt_t = spool.tile([128, 2, NU, NH], FP8, tag="pnt")
        for u in range(NU):
            pt = psmall.tile([128, 32], BF16, tag="psm")
            for i in range(2):
                c = 2 * u + i
                nc.tensor.transpose(pt[:, i * NH:(i + 1) * NH],
                                    pn_t[:, c * 128:(c + 1) * 128],
                                    i16b_t[:])
            nc.vector.tensor_copy(pnt_t[:, :, u, :],
                                  pt[:].rearrange("p (i h) -> p i h", i=2))
        pcx = pwide.tile([128, DIM], F32, tag="wide")
        for h0 in (0, 512):
            for u in range(NU):
                nc.tensor.matmul(pcx[0:NH, h0:h0 + 512],
                                 lhsT=pnt_t[:, :, u, :],
                                 rhs=vw8_t[:, u, :, h0:h0 + 512],
                                 start=(u == 0), stop=(u == NU - 1),
                                 perf_mode=DR)
        # ctx[o] = pcx[head(o), o]: copy to SBUF, transpose 128-col slices,
        # then pick the two half-column blocks (32-aligned partition bases).
        pcs_t = spool.tile([NH, DIM], BF16, tag="pcs")
        nc.scalar.activation(pcs_t[:], pcx[0:NH, :], AF.Identity,
                             scale=S_CX / (S_PN * S_W))
        pe_warm(2)
        cxt_t = spool.tile([128, 2, NU, 4], FP8, tag="cxt")
        nc.vector.memset(cxt_t[:], 0.0)
        if use_vb:
            cxf_t = spool.tile([128, NJ], F32, tag="cxf")
        for u in range(NU):
            pt = psmall.tile([128, 32], BF16, tag="psm")
            for i in range(2):
                c = 2 * u + i
                nc.tensor.transpose(pt[:, i * NH:(i + 1) * NH],
                                    pcs_t[:, c * 128:(c + 1) * 128],
                                    i16b_t[:])
            for i in range(2):
                c = 2 * u + i
                dst_hi = cxf_t[0:64, c:c + 1] if use_vb else cxt_t[0:64, i, u, 0:1]
                dst_lo = cxf_t[64:128, c:c + 1] if use_vb else cxt_t[64:128, i, u, 0:1]
                nc.vector.tensor_copy(dst_hi,
                                      pt[0:64, i * NH + 2 * c:i * NH + 2 * c + 1])
                nc.vector.tensor_copy(dst_lo,
                                      pt[64:128, i * NH + 2 * c + 1:i * NH + 2 * c + 2])
        if use_vb:
            # cxf holds S_CX * ctx; add S_CX * vb then quantize
            vbs_t = spool.tile([128, NJ], F32, tag="vbs")
            nc.vector.tensor_scalar_mul(vbs_t[:], vbt_t[:], S_CX)
            nc.vector.tensor_tensor(cxf_t[:], cxf_t[:], vbs_t[:],
                                    op=AluOpType.add)
            for c in range(NJ):
                nc.vector.tensor_copy(cxt_t[:, c % 2, c // 2, 0:1],
                                      cxf_t[:, c:c + 1])

        # ---------------- out_vec = ow @ ctx + ob --------------------------
        pov = pwide.tile([128, DIM], F32, tag="wide")
        for h0 in (0, 512):
            for u in range(NU):
                nc.tensor.matmul(pov[0:4, h0:h0 + 512],
                                 lhsT=cxt_t[:, :, u, :],
                                 rhs=ow8_t[:, u, :, h0:h0 + 512],
                                 start=(u == 0), stop=(u == NU - 1),
                                 perf_mode=DR)
        pe_warm(2)
        bvec_t = spool.tile([1, DIM], F32, tag="bvec")
        nc.vector.tensor_scalar_mul(bvec_t[:], pov[0:1, :], 1.0 / (S_CX * S_W))
        if use_ob:
            nc.vector.tensor_tensor(bvec_t[:], bvec_t[:], ob_t[:],
                                    op=AluOpType.add)

        # ---------------- residual + LayerNorm -----------------------------
        # h = x + v (v = bvec broadcast over rows).  Per row s:
        #   mu_h[s]  = mu_x[s] + mu_v
        #   var_h[s] = Sxx[s]/D - mu_x^2 + var_v  (+ 2*cov(x_row, v))
        # The cov term is O(sigma_x*sigma_v/sqrt(D)) ~ 4e-4 of var_h and is
        # dropped: well inside the 2e-2 gate.  mu_x comes from the scores
        # matmul's folded ones-row; Sxx from the early square passes, so
        # after bvec only tiny [128, NT] fixups remain before the tile loop.
        bvr16_t = spool.tile([1, DIM], BF16, tag="bvr16")
        nc.vector.tensor_copy(bvr16_t[:], bvec_t[:])
        if use_g:
            gb_t = spool.tile([128, DIM], F32, tag="gb")
            bcast_row(g_t, gb_t, 128)
        if use_b:
            bb_t = spool.tile([128, DIM], F32, tag="bb")
            bcast_row(b_t, bb_t, 128)

        # bvb = v broadcast over partitions, via PE rank-1 + ACT copy
        pbv = pwide.tile([128, DIM], F32, tag="wide")
        for h0 in (0, 512):
            nc.tensor.matmul(pbv[:, h0:h0 + 512], lhsT=ones1x128b[:],
                             rhs=bvr16_t[0:1, h0:h0 + 512],
                             start=True, stop=True)
        bvb_t = spool.tile([128, DIM], BF16, tag="bvb")
        nc.scalar.activation(bvb_t[:], pbv[:], AF.Identity)

        # scalars of v: sv = [mu_v, var_v] -> bsc [128, 2] via PE rank-1
        sv_t = spool.tile([1, 2], F32, tag="sv")
        nc.vector.reduce_sum(sv_t[0:1, 0:1], bvec_t[:], AX.X)
        junk1_t = spool.tile([1, DIM], F32, tag="junk1")
        nc.vector.scalar_tensor_tensor(junk1_t[:], bvec_t[:], 1.0, bvec_t[:],
                                       AluOpType.mult, AluOpType.mult,
                                       accum_out=sv_t[0:1, 1:2])
        nc.vector.tensor_scalar_mul(sv_t[:], sv_t[:], 1.0 / DIM)
        muv2_t = spool.tile([1, 1], F32, tag="muv2")
        nc.vector.tensor_tensor(muv2_t[:], sv_t[0:1, 0:1], sv_t[0:1, 0:1],
                                op=AluOpType.mult)
        nc.vector.tensor_tensor(sv_t[0:1, 1:2], sv_t[0:1, 1:2], muv2_t[:],
                                op=AluOpType.subtract)
        pbs = psmall.tile([128, 32], F32, tag="psm")
        nc.tensor.matmul(pbs[:, 0:2], lhsT=ones1x128[:], rhs=sv_t[:],
                         start=True, stop=True)
        bsc_t = spool.tile([128, 2], F32, tag="bsc")
        nc.vector.tensor_copy(bsc_t[:], pbs[:, 0:2])

        # batched fixups over [128, NT]
        mux_t = spool.tile([128, NT], F32, tag="mux")
        nc.vector.tensor_scalar_mul(mux_t[:], rsc_t[:], 1.0 / DIM)
        mx2_t = spool.tile([128, NT], F32, tag="mx2")
        nc.vector.tensor_tensor(mx2_t[:], mux_t[:], mux_t[:],
                                op=AluOpType.mult)
        varh_t = spool.tile([128, NT], F32, tag="varh")
        nc.vector.scalar_tensor_tensor(varh_t[:], ssq_t[:], 1.0 / DIM,
                                       mx2_t[:], AluOpType.mult,
                                       AluOpType.subtract)
        nc.vector.tensor_scalar(varh_t[:], varh_t[:], bsc_t[:, 1:2], LN_EPS,
                                AluOpType.add, AluOpType.add)
        rv_t = spool.tile([128, NT], F32, tag="rv")
        nc.vector.reciprocal(rv_t[:], varh_t[:])
        rstd_t = spool.tile([128, NT], F32, tag="rstd")
        nc.scalar.sqrt(rstd_t[:], rv_t[:])
        muh_t = spool.tile([128, NT], F32, tag="muh")
        nc.vector.tensor_scalar(muh_t[:], mux_t[:], bsc_t[:, 0:1], None,
                                AluOpType.add)
        nmrw_t = spool.tile([128, NT], F32, tag="nmrw")
        nc.vector.scalar_tensor_tensor(nmrw_t[:], muh_t[:], -1.0, rstd_t[:],
                                       AluOpType.mult, AluOpType.mult)

        # per tile: DVE h-add, ACT/DVE normalize, DMA out
        for t in range(NT):
            t1_t = jpool.tile([128, DIM], BF16, tag="h")
            nc.vector.tensor_tensor(t1_t[:], xbn_t[:, t, :], bvb_t[:],
                                    op=AluOpType.add)
            o_t = opool.tile([128, DIM], BF16, tag="o")
            if t % 3 != 2:
                nc.scalar.activation(o_t[:], t1_t[:], AF.Identity,
                                     bias=nmrw_t[:, t:t + 1],
                                     scale=rstd_t[:, t:t + 1])
            else:
                nc.vector.tensor_scalar(o_t[:], t1_t[:], rstd_t[:, t:t + 1],
                                        nmrw_t[:, t:t + 1], AluOpType.mult,
                                        AluOpType.add)
            if use_g:
                nc.vector.tensor_tensor(o_t[:], o_t[:], gb_t[:],
                                        op=AluOpType.mult)
            if use_b:
                nc.vector.tensor_tensor(o_t[:], o_t[:], bb_t[:],
                                        op=AluOpType.add)
            nc.sync.dma_start(out_d[t * 128:(t + 1) * 128, :], o_t[:])

    nc.compile()
    return nc


def _get_program(flags):
    if flags not in _cache:
        _cache[flags] = _build(flags)
    return _cache[flags]


def _dr_pack(a):
    """[1024, W] -> [128, 4*2*W] DoubleRow-interleaved SBUF image."""
    w = a.shape[1]
    return np.ascontiguousarray(
        a.reshape(NU, 2, 128, w).transpose(2, 0, 1, 3).reshape(128, NU * 2 * w))


def build_in_maps(inputs):
    f32 = lambda a: np.ascontiguousarray(np.asarray(a, np.float32))
    bf = ml_dtypes.bfloat16
    f8 = ml_dtypes.float8_e4m3
    x = f32(inputs["x"])
    xnb = f32(inputs["x_neighbor"])
    mask = np.ascontiguousarray(np.asarray(inputs["mask"], np.int32))
    nmask = f32(inputs["neighbor_mask"])
    qw, qb = f32(inputs["qw"]), f32(inputs["qb"])
    kw, kb = f32(inputs["kw"]), f32(inputs["kb"])
    vw, vb = f32(inputs["vw"]), f32(inputs["vb"])
    ow, ob = f32(inputs["ow"]), f32(inputs["ob"])
    ln_g, ln_b = f32(inputs["ln_g"]), f32(inputs["ln_b"])

    flags = (bool(qb.any()), bool(kb.any()), bool(vb.any()), bool(ob.any()),
             bool((ln_g != 1.0).any()), bool(ln_b.any()),
             bool((mask == 0).any()))
    use_qb, use_kb, use_vb, use_ob, use_g, use_b, use_mask = flags

    qw8 = _dr_pack((S_W * qw.T).astype(f8))
    kw8 = _dr_pack((S_W * kw).astype(f8))
    vw8 = _dr_pack((S_W * vw.T).astype(f8))
    ow8 = _dr_pack((S_W * ow.T).astype(f8))
    ident = np.zeros((16, 48), np.float32)
    ident[:, 0:16] = np.eye(16)
    ident[:, 16:32] = np.eye(16)
    ident[:, 32:48] = np.eye(16)
    i128b = np.eye(128, dtype=bf)

    in_maps = []
    for b in range(BS):
        m = {
            "qw8": qw8, "kw8": kw8, "vw8": vw8, "ow8": ow8,
            "xt8": _dr_pack(np.ascontiguousarray(x[b].T).astype(f8)),
            "xbn": np.ascontiguousarray(
                x[b].reshape(NT, 128, DIM).transpose(1, 0, 2)
                .reshape(128, NT * DIM)).astype(bf),
            "xnb": np.ascontiguousarray(xnb[b]).astype(bf),
            "nmp": np.ascontiguousarray(nmask[b]).astype(bf),
            "nmr": np.ascontiguousarray(nmask[b]),
            "ident": ident,
            "i128b": i128b,
        }
        if use_qb:
            m["qb"] = qb
        if use_kb:
            m["kbt"] = np.ascontiguousarray(kb.reshape(NJ, 128).T).astype(bf)
        if use_vb:
            m["vbt"] = np.ascontiguousarray(vb.reshape(NJ, 128).T)
        if use_ob:
            m["ob"] = ob
        if use_g:
            m["lng"] = ln_g
        if use_b:
            m["lnb"] = ln_b
        if use_mask:
            m["mask"] = np.ascontiguousarray(mask[b])
        in_maps.append(m)
    return flags, in_maps


def kernel(**inputs):
    flags, in_maps = build_in_maps(inputs)
    nc = _get_program(flags)
    res = bass_utils.run_bass_kernel_spmd(nc, in_maps,
                                          core_ids=list(range(N_CORES)))
    return np.stack([res.results[b]["out"] for b in range(BS)]).astype(np.float32)
